# revision 29
# baseline (speedup 1.0000x reference)
"""Block-sparse linear layer (x @ (mask*W).T + bias) on 8 TRN2 NeuronCores.

Strategy: data-parallel over batch rows. Each core gets 1024 rows of x
(transposed to [k, m] on host, cast to bf16), the packed kept weight
blocks (bf16), and bias. On-device: out.T tile [o=128, m=1024] accumulates
in PSUM over the 16 kept k-subtiles (k-subtile = 128 rows), with W tiles
stationary and x slabs moving. PSUM is evicted through the vector/scalar
engines with the per-partition bias add fused, then DMA'd out. The host
reassembles the full [8192, 4096] fp32 output.
"""

import sys
import types

import numpy as np
import ml_dtypes

BATCH = 8192
SIZE = 4096
NB = 16
BLOCK = 256
NCORES = 8
MC = BATCH // NCORES  # 1024 rows per core
P = 128
KS = SIZE // P  # 32 k-subtiles
OT = SIZE // P  # 32 o-tiles
MM_N = 512  # moving free dim per matmul

_BUILD_CACHE = {}


def _install_ntff_hook():
    # Register the axon NTFF profiling hook if the image's antenv lacks it.
    if "antenv.axon_hooks" in sys.modules:
        return
    try:
        from trn_agent_boot.trn_boot import _ntff_profile_via_ctypes

        hook = _ntff_profile_via_ctypes("/opt/axon/libaxon_pjrt.so")
        mod = types.ModuleType("antenv.axon_hooks")
        mod.get_axon_ntff_profile_hook = lambda: hook
        sys.modules["antenv.axon_hooks"] = mod
    except Exception:
        pass


def _block_keep_from_mask(mask):
    """Return [NB, NB] bool of kept blocks if mask is block-constant, else None."""
    m4 = np.asarray(mask).reshape(NB, BLOCK, NB, BLOCK)
    keep = m4[:, 0, :, 0]
    uniform = np.all(m4 == keep[:, None, :, None])
    return keep if uniform else None


def _ks_lists(keep):
    """Per o-tile (128 outputs) list of kept k-subtile indices, padded to
    a uniform length (padding points at subtile 0 with zero weights)."""
    lists = []
    for t in range(OT):
        i = (t * P) // BLOCK  # o-block row
        ks = []
        for j in range(NB):
            if keep[i, j]:
                base = (j * BLOCK) // P
                ks.extend(range(base, base + BLOCK // P))
        lists.append(ks)
    n_sub = max(1, max(len(l) for l in lists))
    padded = tuple(tuple(l + [-1] * (n_sub - len(l))) for l in lists)
    return padded, n_sub


def _build(ks_lists, n_sub):
    import concourse.mybir as mybir
    import concourse.tile as tile
    from concourse import bacc

    bf16, f32 = mybir.dt.bfloat16, mybir.dt.float32
    nc = bacc.Bacc("TRN2", target_bir_lowering=False)
    xt_d = nc.declare_dram_parameter("xt", [P, KS, MC], bf16, isOutput=False)
    wt_d = nc.declare_dram_parameter("wt", [OT, P, n_sub, P], bf16, isOutput=False)
    bias_d = nc.declare_dram_parameter("biast", [P, OT], f32, isOutput=False)
    out_d = nc.declare_dram_parameter("out", [OT, P, MC], f32, isOutput=True)

    # x DMA issue order: k-subtiles in order of first use across o-tiles.
    ks_order = []
    for t in range(OT):
        for ks in ks_lists[t]:
            if ks >= 0 and ks not in ks_order:
                ks_order.append(ks)
    for ks in range(KS):
        if ks not in ks_order:
            ks_order.append(ks)

    W_POOL_BUFS = 8
    XG = 2  # x chunks per DMA group

    with tile.TileContext(nc) as tc:
        with (
            tc.tile_pool(name="const", bufs=1) as const_pool,
            tc.tile_pool(name="xpool", bufs=1) as xpool,
            tc.tile_pool(name="wpool", bufs=W_POOL_BUFS) as wpool,
            tc.tile_pool(name="opool", bufs=3) as opool,
            tc.tile_pool(name="psum", bufs=4, space="PSUM") as psum_pool,
        ):
            bias_tile = const_pool.tile([P, OT], f32)
            nc.gpsimd.dma_start(out=bias_tile[:], in_=bias_d[:])

            # Warm the PE clock (HAM un-throttles after ~3.4us of sustained
            # matmul activity) with dummy matmuls on zeroed SBUF while the
            # first x/W DMAs are still in flight.
            warm = const_pool.tile([P, MM_N], bf16, name="warm")
            nc.vector.memset(warm[:], 0)
            warm_ps = psum_pool.tile([P, MM_N], f32, name="warm_ps", tag="ps")
            N_WARM = 8
            for i in range(N_WARM):
                nc.tensor.matmul(
                    warm_ps[:],
                    lhsT=warm[:, 0:P],
                    rhs=warm[:],
                    start=(i == 0),
                    stop=(i == N_WARM - 1),
                )

            w_tiles = {}

            def w_alloc(t):
                w_tiles[t] = wpool.tile([P, n_sub, P], bf16, name="w_tile")

            def w_dma(t, engine, half=None):
                if t not in w_tiles:
                    w_alloc(t)
                w = w_tiles[t]
                if half is None:
                    lo, hi = 0, n_sub
                else:
                    step = (n_sub + 1) // 2
                    lo, hi = half * step, min((half + 1) * step, n_sub)
                engine.dma_start(
                    out=w[:, lo:hi, :], in_=wt_d[t, :, lo:hi, :]
                )

            x_ap = {}
            x_groups = []
            for gstart in range(0, len(ks_order), XG):
                grp = ks_order[gstart : gstart + XG]
                lo = min(grp)
                assert grp == list(range(lo, lo + len(grp))), grp
                x_groups.append((lo, len(grp)))

            def x_dma(gi):
                lo, n = x_groups[gi]
                xg = xpool.tile([P, n, MC], bf16, name=f"x_g{gi}", uniquify=False)
                nc.sync.dma_start(out=xg[:], in_=xt_d[:, lo : lo + n, :])
                for off in range(n):
                    x_ap[lo + off] = xg[:, off, :]

            # Startup-critical DMAs on the fast Sync queue, ordered to match
            # PE consumption (the first FOUR o-tiles run interleaved
            # chunk-major, so all four weight-tile first-halves lead).
            # Later W tiles stream on the GpSimd queue.
            for t in range(4):
                w_dma(t, nc.sync, half=0)
            x_dma(0)
            x_dma(1)
            w_dma(2, nc.sync, half=1)
            w_dma(3, nc.sync, half=1)
            x_dma(2)
            x_dma(3)
            w_dma(0, nc.sync, half=1)
            w_dma(1, nc.sync, half=1)
            for gi in range(4, 9):
                x_dma(gi)
            w_dma(4, nc.sync)
            w_dma(5, nc.sync)
            for gi in range(9, len(x_groups)):
                x_dma(gi)

            def emit_block(ts, interleave):
                """Emit the accumulation + eviction for o-tiles `ts`.

                interleave=True: chunk-major across the tiles (each arriving
                x chunk is consumed by every tile that uses it — PE executes
                strictly in order, so this is what absorbs DMA latency).
                interleave=False: tile-major (first tile finishes early so
                its eviction overlaps the next tile's matmuls).
                """
                ps = {t: psum_pool.tile([P, MC], f32, name="ps") for t in ts}
                sets = {t: {ks: s for s, ks in enumerate(ks_lists[t]) if ks >= 0} for t in ts}
                for t in ts:
                    if not sets[t]:  # fully-masked o-tile: zero the PSUM
                        sets[t] = {ks_order[0]: 0}
                n_done = {t: 0 for t in ts}
                if interleave:
                    order = [
                        (c, t)
                        for c in ks_order
                        for t in ts
                        if c in sets[t]
                    ]
                else:
                    order = [(c, t) for t in ts for c in ks_lists[t] if c >= 0]
                for c, t in order:
                    s = sets[t][c]
                    first = n_done[t] == 0
                    n_done[t] += 1
                    last = n_done[t] == len(sets[t])
                    for h in range(MC // MM_N):
                        nc.tensor.matmul(
                            ps[t][:, h * MM_N : (h + 1) * MM_N],
                            lhsT=w_tiles[t][:, s, :],
                            rhs=x_ap[c][:, h * MM_N : (h + 1) * MM_N],
                            start=first,
                            stop=last,
                        )
                    if not interleave and last:
                        _evict(ts, t, ps)
                if interleave:
                    for t in ts:
                        _evict(ts, t, ps)

            def _evict(ts, t, ps):
                # Evict in halves (out-DMA of the first half overlaps the
                # bias-add of the second); even o-tiles on the Vector
                # engine, odd on Scalar, so neighbor evictions parallelize.
                o_tile = opool.tile([P, MC], f32, name="o_tile")
                half = MC // 2
                for h in range(2):
                    sl = slice(h * half, (h + 1) * half)
                    if t % 2 == 0:
                        nc.vector.tensor_scalar_add(
                            o_tile[:, sl], ps[t][:, sl], bias_tile[:, t : t + 1]
                        )
                    else:
                        nc.scalar.add(
                            o_tile[:, sl], ps[t][:, sl], bias_tile[:, t : t + 1]
                        )
                    nc.sync.dma_start(out=out_d[t, :, sl], in_=o_tile[:, sl])

            # First four o-tiles as one interleaved block (their k-chunk
            # sets overlap heavily, maximizing PE work per arriving byte
            # during the x load); middle o-tiles pair-wise; last pair
            # tile-major so the final evictions overlap compute.
            emit_block((0, 1, 2, 3), interleave=True)
            for pair in range(2, OT // 2):
                ts = (2 * pair, 2 * pair + 1)
                for t in ts:
                    if t >= 6:
                        w_dma(t, nc.gpsimd)
                emit_block(ts, interleave=(pair != OT // 2 - 1))
    nc.compile()
    return nc


def _get_kernel(ks_lists, n_sub):
    key = (ks_lists, n_sub)
    if key not in _BUILD_CACHE:
        _BUILD_CACHE[key] = _build(ks_lists, n_sub)
    return _BUILD_CACHE[key]


def kernel(x, weight, bias, mask, _trace=False):
    from concourse.bass_utils import run_bass_kernel_spmd

    _install_ntff_hook()

    x = np.asarray(x)
    weight = np.asarray(weight)
    bias = np.asarray(bias, dtype=np.float32)
    keep = _block_keep_from_mask(mask)
    if keep is None:
        # Mask not block-constant: fall back to a dense schedule with the
        # element-masked weights and every k-subtile kept.
        weight = np.where(np.asarray(mask), weight, 0.0).astype(np.float32)
        keep = np.ones((NB, NB), dtype=bool)
    ks_lists, n_sub = _ks_lists(keep)

    nc = _get_kernel(ks_lists, n_sub)

    # Pack weights: wt[t, p, s, q] = W[t*P + q, ks*P + p] for kept subtile ks.
    w4 = weight.reshape(OT, P, KS, P)  # [t, q, ks, p]
    wt = np.zeros((OT, P, n_sub, P), dtype=ml_dtypes.bfloat16)
    for t in range(OT):
        idx = [ks for ks in ks_lists[t]]
        valid = [s for s, ks in enumerate(idx) if ks >= 0]
        sel = w4[t][:, [idx[s] for s in valid], :]  # [q, s_valid, p]
        wt[t][:, valid, :] = sel.transpose(2, 1, 0).astype(ml_dtypes.bfloat16)

    biast = np.ascontiguousarray(
        bias.reshape(OT, P).T, dtype=np.float32
    )  # [P, OT]

    in_maps = []
    for c in range(NCORES):
        xc = x[c * MC : (c + 1) * MC, :]  # [MC, SIZE] fp32
        xt = np.ascontiguousarray(
            xc.reshape(MC, KS, P).transpose(2, 1, 0)
        ).astype(ml_dtypes.bfloat16)  # [P, KS, MC]
        in_maps.append({"xt": xt, "wt": wt, "biast": biast})

    res = run_bass_kernel_spmd(nc, in_maps, list(range(NCORES)), trace=_trace)

    out = np.empty((BATCH, SIZE), dtype=np.float32)
    for c in range(NCORES):
        o = res.results[c]["out"]  # [OT, P, MC]
        out[c * MC : (c + 1) * MC, :] = o.reshape(SIZE, MC).T
    if _trace:
        return out, res
    return out
